# revision 7
# baseline (speedup 1.0000x reference)
"""Complementary gray-code structured-light decoder on 8 Trainium2 NeuronCores.

kernel(images: [24, 2048, 2448] f32) -> [2048, 2448, 2] f32

Sharding: H rows split across 8 cores (256 rows each), data-parallel, no
cross-core communication. Per core the 256x2448 slab is processed as 12
tiles of [128 rows x 408 cols].

V2 rework of the validated baseline (sim-checked: rel_l2 2.6e-4 on CPU):
  - s, c drop the f32-einsum EPS emulation: plain subtracts (GPSIMD).
  - threshold: pairwise f32 tree (GPSIMD+DVE) instead of 8 fp32 PE matmuls;
    compares test (8*gc > sum8) via one fused scalar_tensor_tensor so the
    /8 scale op disappears.
  - reciprocal: DVE reciprocal_approx_fast (1 op) instead of full-precision
    iterative reciprocal (5.2us/tile).
  - arctan output, sela, sign planes in bf16 -> every t4 matmul is bf16.
  - XOR cascade on int32 bitcast of the {0,1.0} bf16 planes (bitwise_xor
    on 0x3F80 patterns == logical_xor, half the DVE elements).
  - final combine fused: out = (t4 + 4) * mask in one scalar_tensor_tensor
    reading PSUM (drops the +4*ones matmuls).

Math per pixel (both directions d):
  s = i1-i3, c = i0-i2 ; thr8 = treesum(i0..i7) ; bit_i = (8*gc_i > thr8)
  x_i = bit_0 ^..^ bit_i ; q = s^2+c^2 ; mask = max(q_col,q_row) > T_EFF
  u = s*c/max(s^2,c^2) ; sel = s^2>c^2 ; t4 = -S*atan(u) + 2S*sel*atan(u)
      - 4*sel*sgn(u) + sum_i 2^(10-i) x_i + 8 x_7 + 8 x_7 sgn(c) - 4 sgn(c)
  out_d = (t4 + 4) * mask * 1   [S = 16/2pi; collapsed unwrap identity]
"""
import numpy as np

import concourse.bass as bass
import concourse.mybir as mybir
import concourse.tile as tile
from concourse.vector_clock import ScopedClock
from concourse.bass_utils import run_bass_kernel_spmd

# ---------------- constants ----------------
H, W = 2048, 2448
NFRAMES = 24
NCORES = 8
ROWS_PER_CORE = H // NCORES          # 256
F = 408                              # tile free width; W = 6*F
T_EFF = 0.010000010952353477         # (q > T_EFF) == (0.5*sqrt(q) > 0.05f)
SCALE = float(np.float32(16.0 / (2.0 * np.float64(np.pi))))
WB_VALS = [-SCALE, 2.0 * SCALE, 1024.0, 512.0, 256.0, 128.0, 64.0, 32.0,
           16.0, 8.0, -4.0]
W_NSC, W_2SC = 0, 1
W_XI0 = 2                            # slots 2..8 hold 1024..16 for x_0..x_6
W_P8, W_N4 = 9, 10

f32 = mybir.dt.float32
bf16 = mybir.dt.bfloat16
i32 = mybir.dt.int32
OP = mybir.AluOpType
AF = mybir.ActivationFunctionType

_ctr = [0]


def _sanitize_waits(nc):
    """This walrus build rejects instructions carrying >1 sync wait. Move
    excess waits onto fresh same-engine NOPs inserted just before."""
    for f in nc.m.functions:
        for bb in f.blocks:
            il = bb.instructions
            i = 0
            while i < len(il):
                ins = il[i]
                si = getattr(ins, "sync_info", None)
                waits = list(si.on_wait) if si is not None and si.on_wait else []
                if len(waits) > 1:
                    si.on_wait = [waits[-1]]
                    ins.sync_info = si
                    for w in waits[:-1]:
                        _ctr[0] += 1
                        n = mybir.InstNoOp(name=f"waitsplit_{_ctr[0]}")
                        n.engine = ins.engine
                        n.sync_info = mybir.SyncInfo(on_wait=[w], on_update=[])
                        il.insert(i, n)
                        i += 1
                i += 1


class _SafeTileContext(tile.TileContext):
    """TileContext whose exit drain splits its sem waits across SP NOPs
    (the drain is emitted inside __exit__, before _sanitize_waits can run)."""

    def _drain_and_barrier(self, tick_clock, wait_clock):
        nop_inst = self.nc.sync.nop()
        wait_clock.add_sem_waits(
            nop_inst.ins, ScopedClock({None: tick_clock.global_clock})
        )
        si = nop_inst.ins.sync_info
        waits = list(si.on_wait) if si is not None else []
        if len(waits) > 1:
            si.on_wait = waits[:1]
            nop_inst.ins.sync_info = si
            for w in waits[1:]:
                n2 = self.nc.sync.nop()
                n2.ins.sync_info = mybir.SyncInfo(on_wait=[w], on_update=[])
        self.nc.sync.drain()

        self.nc.all_engine_barrier()
        assert self.sems is not None
        popped = self.nc._tile_sem_poison_stack.pop()
        assert popped is self._sem_poison
        self.nc.clear_and_free_semaphores(list(self.sems.allocated().values()))
        self.nc.all_engine_barrier()


def _build_program(sanitize=True):
    import contextlib

    nc = bass.Bass("TRN2", target_bir_lowering=False, debug=False)
    img = nc.dram_tensor("img", [NFRAMES, ROWS_PER_CORE, W], f32, kind="ExternalInput")
    wtsb = nc.dram_tensor("wtsb", [len(WB_VALS), 128, 128], bf16, kind="ExternalInput")
    out = nc.dram_tensor("out", [ROWS_PER_CORE, W, 2], f32, kind="ExternalOutput")

    with _SafeTileContext(nc) as tc, contextlib.ExitStack() as ctx:
        wpool = ctx.enter_context(tc.tile_pool(name="wpool", bufs=1))
        inp = ctx.enter_context(tc.tile_pool(name="inp", bufs=2))
        sb = ctx.enter_context(tc.tile_pool(name="sb", bufs=1))
        outp = ctx.enter_context(tc.tile_pool(name="outp", bufs=2))
        ps_t = ctx.enter_context(tc.tile_pool(name="ps_t", bufs=2, space="PSUM"))

        wtb = wpool.tile([128, len(WB_VALS) * 128], bf16, tag="wtb")
        for wi in range(len(WB_VALS)):
            nc.sync.dma_start(out=wtb[:, wi * 128:(wi + 1) * 128], in_=wtsb[wi, :, :])

        def wb(i):
            return wtb[:, i * 128:(i + 1) * 128]

        for rb in range(ROWS_PER_CORE // 128):
            r0 = rb * 128
            for cb in range(W // F):
                c0 = cb * F
                # ---------------- load ----------------
                X = inp.tile([128, NFRAMES * F], f32, tag="X")
                nc.sync.dma_start(
                    out=X[:, :].rearrange("p (f x) -> p f x", f=NFRAMES),
                    in_=img[:, r0:r0 + 128, c0:c0 + F].rearrange("f p x -> p f x"),
                )
                # PS frames as [2, 4, F]: [:, j, :] = frame-pair (j, j+4)
                Xp = X[:, 0:8 * F].rearrange("p (a b x) -> p a b x", a=2, b=4)
                # gray frames as [d, bit, x]
                Xg = X[:, 8 * F:24 * F].rearrange("p (d f x) -> p d f x", d=2, f=8)

                # ---------------- s, c (GPSIMD) ----------------
                # cs layout: [c_col | s_col | c_row | s_row]
                cs = sb.tile([128, 4 * F], f32, tag="cs", bufs=2)
                csv = cs[:, :].rearrange("p (g t y) -> p g t y", g=2, t=2)
                c_view = csv[:, :, 0, :]
                s_view = csv[:, :, 1, :]
                nc.gpsimd.tensor_tensor(c_view, Xp[:, :, 0, :], Xp[:, :, 2, :], OP.subtract)
                nc.gpsimd.tensor_tensor(s_view, Xp[:, :, 1, :], Xp[:, :, 3, :], OP.subtract)

                # ---------------- threshold tree ----------------
                P4 = sb.tile([128, 4 * F], f32, tag="P4")
                nc.gpsimd.tensor_tensor(P4[:, :], X[:, 0:4 * F], X[:, 4 * F:8 * F], OP.add)
                Q2 = sb.tile([128, 2 * F], f32, tag="Q2")
                nc.vector.tensor_tensor(Q2[:, :], P4[:, 0:2 * F], P4[:, 2 * F:4 * F], OP.add)
                thr8 = sb.tile([128, F], f32, tag="thr8", bufs=2)
                nc.vector.tensor_tensor(thr8[:, :], Q2[:, 0:F], Q2[:, F:2 * F], OP.add)
                thr8b = thr8[:, :].rearrange("p (o v x) -> p o v x", o=1, v=1)

                # ---------------- gray compares (fused scale) ----------------
                # xbits layout: [x0 pair | x7 pair | sel pair]  (pair = col,row)
                xbits = sb.tile([128, 6 * F], bf16, tag="xbits", bufs=2)
                nc.vector.scalar_tensor_tensor(
                    xbits[:, 0:2 * F].rearrange("p (d x) -> p d x", d=2),
                    Xg[:, :, 0, :], 8.0,
                    thr8[:, :].rearrange("p (o x) -> p o x", o=1).broadcast_to([128, 2, F]),
                    OP.mult, OP.is_gt,
                )
                b_rest = sb.tile([128, 14 * F], bf16, tag="b_rest")
                brv = b_rest[:, :].rearrange("p (f d x) -> p d f x", f=7, d=2)
                for d in range(2):
                    nc.vector.scalar_tensor_tensor(
                        brv[:, d, :, :], Xg[:, d, 1:8, :], 8.0,
                        thr8b[:, 0, :, :].broadcast_to([128, 7, F]),
                        OP.mult, OP.is_gt,
                    )

                # ---------------- XOR cascade (int32 alias) ----------------
                x_all = sb.tile([128, 12 * F], bf16, tag="x_all", bufs=2)

                def xpair(i):  # contiguous [128, 2F] cumulative bit i (pair)
                    if i == 0:
                        return xbits[:, 0:2 * F]
                    if i == 7:
                        return xbits[:, 2 * F:4 * F]
                    return x_all[:, (i - 1) * 2 * F:i * 2 * F]

                for i in range(1, 8):
                    nc.vector.tensor_tensor(
                        xpair(i).bitcast(i32), xpair(i - 1).bitcast(i32),
                        b_rest[:, (i - 1) * 2 * F:i * 2 * F].bitcast(i32),
                        OP.bitwise_xor,
                    )

                def x_dir(i, d):
                    return xpair(i)[:, d * F:(d + 1) * F]

                # ---------------- squares, magnitude mask ----------------
                sq_s = sb.tile([128, 2 * F], f32, tag="sq_s")
                sq_c = sb.tile([128, 2 * F], f32, tag="sq_c")
                nc.scalar.activation(
                    sq_s[:, :].rearrange("p (d x) -> p d x", d=2),
                    s_view, AF.Square, bias=0.0, scale=1.0,
                )
                nc.scalar.activation(
                    sq_c[:, :].rearrange("p (d x) -> p d x", d=2),
                    c_view, AF.Square, bias=0.0, scale=1.0,
                )
                q_all = sb.tile([128, 2 * F], f32, tag="q_all")
                nc.gpsimd.tensor_tensor(q_all[:, :], sq_s[:, :], sq_c[:, :], OP.add)
                qm = sb.tile([128, F], f32, tag="qm")
                nc.vector.tensor_tensor(qm[:, :], q_all[:, 0:F], q_all[:, F:2 * F], OP.max)
                mask = sb.tile([128, F], f32, tag="mask", bufs=2)
                nc.vector.tensor_single_scalar(mask[:, :], qm[:, :], T_EFF, OP.is_gt)

                # ---------------- bounded atan path ----------------
                ma2 = sb.tile([128, 2 * F], f32, tag="ma2")
                nc.vector.tensor_tensor(ma2[:, :], sq_s[:, :], sq_c[:, :], OP.max)
                p_sc = sb.tile([128, 2 * F], f32, tag="p_sc")
                nc.gpsimd.tensor_tensor(
                    p_sc[:, :].rearrange("p (d x) -> p d x", d=2),
                    s_view, c_view, OP.mult,
                )
                rcm = sb.tile([128, 2 * F], f32, tag="rcm")
                nc.vector.reciprocal(rcm[:, :], ma2[:, :])
                u_all = sb.tile([128, 2 * F], f32, tag="u_all")
                nc.gpsimd.tensor_tensor(u_all[:, :], p_sc[:, :], rcm[:, :], OP.mult)
                a_u = sb.tile([128, 2 * F], bf16, tag="a_u", bufs=2)
                nc.scalar.activation(a_u[:, :], u_all[:, :], AF.Arctan, bias=0.0, scale=1.0)

                # sgn layout: [sgn_c pair | sgn_u pair]
                sgn = sb.tile([128, 4 * F], bf16, tag="sgn", bufs=2)
                nc.scalar.activation(
                    sgn[:, 0:2 * F].rearrange("p (d x) -> p d x", d=2),
                    c_view, AF.Sign, bias=0.0, scale=1.0,
                )
                nc.scalar.activation(sgn[:, 2 * F:4 * F], p_sc[:, :], AF.Sign, bias=0.0, scale=1.0)

                # sel into xbits[4F:6F]; then sgnm = [x7*sgn_c | sel*sgn_u]
                nc.vector.tensor_tensor(xbits[:, 4 * F:6 * F], sq_s[:, :], sq_c[:, :], OP.is_gt)
                sela = sb.tile([128, 2 * F], bf16, tag="sela", bufs=2)
                nc.vector.tensor_tensor(sela[:, :], xbits[:, 4 * F:6 * F], a_u[:, :], OP.mult)
                sgnm = sb.tile([128, 4 * F], bf16, tag="sgnm", bufs=2)
                nc.vector.tensor_tensor(
                    sgnm[:, :],
                    xbits[:, 2 * F:6 * F].rearrange("p (h y) -> p h y", h=2),
                    sgn[:, :].rearrange("p (h y) -> p h y", h=2),
                    OP.mult,
                )

                # ---------------- PE: t4 linear combine (all bf16) ----------
                # t4 = -S*a_u + 2S*sela - 4*sel*sgn_u + sum_i 2^(10-i)*x_i
                #      + 8*x7 + 8*x7*sgn_c - 4*sgn_c     (+4 fused into output)
                t4 = ps_t.tile([128, 1024], f32, tag="t4", name="t4")
                for d in range(2):
                    sl = slice(d * F, (d + 1) * F)
                    pd = t4[:, d * 512:d * 512 + F]
                    nc.tensor.matmul(pd, wb(W_NSC), a_u[:, sl], start=True, stop=False)
                    nc.tensor.matmul(pd, wb(W_2SC), sela[:, sl], start=False, stop=False)
                    nc.tensor.matmul(pd, wb(W_N4), sgnm[:, 2 * F + d * F:2 * F + (d + 1) * F], start=False, stop=False)
                    for i in range(7):
                        nc.tensor.matmul(pd, wb(W_XI0 + i), x_dir(i, d), start=False, stop=False)
                    nc.tensor.matmul(pd, wb(W_P8), x_dir(7, d), start=False, stop=False)
                    nc.tensor.matmul(pd, wb(W_P8), sgnm[:, sl], start=False, stop=False)
                    nc.tensor.matmul(pd, wb(W_N4), sgn[:, sl], start=False, stop=True)

                # ---------------- fused (t4+4)*mask + interleaved store -----
                o_t = outp.tile([128, F * 2], f32, tag="o_t")
                ov = o_t[:, :].rearrange("p (x two) -> p two x", two=2)
                t4v = t4[:, :].rearrange("p (g y) -> p g y", g=2)[:, :, 0:F]
                maskb = mask[:, :].rearrange("p (o x) -> p o x", o=1).broadcast_to([128, 2, F])
                nc.vector.scalar_tensor_tensor(
                    ov, t4v, 4.0, maskb, OP.add, OP.mult,
                )
                nc.sync.dma_start(
                    out=out[r0:r0 + 128, c0:c0 + F, :].rearrange("p x two -> p (x two)"),
                    in_=o_t[:, :],
                )

    if sanitize:
        _sanitize_waits(nc)
    return nc


def _weights_b():
    import ml_dtypes
    I = np.eye(128, dtype=np.float32)
    return np.stack([np.float32(v) * I for v in WB_VALS]).astype(ml_dtypes.bfloat16)


_CACHE = {}


def _in_maps(images):
    wtsb = _weights_b()
    maps = []
    for core in range(NCORES):
        r0 = core * ROWS_PER_CORE
        maps.append({
            "img": np.ascontiguousarray(images[:, r0:r0 + ROWS_PER_CORE, :]),
            "wtsb": wtsb,
        })
    return maps


def kernel(images: np.ndarray) -> np.ndarray:
    images = np.ascontiguousarray(np.asarray(images, dtype=np.float32))
    assert images.shape == (NFRAMES, H, W), images.shape
    if "nc" not in _CACHE:
        _CACHE["nc"] = _build_program()
    res = run_bass_kernel_spmd(_CACHE["nc"], _in_maps(images), core_ids=list(range(NCORES)))
    out = np.empty((H, W, 2), dtype=np.float32)
    for core in range(NCORES):
        r0 = core * ROWS_PER_CORE
        out[r0:r0 + ROWS_PER_CORE] = res.results[core]["out"]
    return out


def timed_run(images: np.ndarray):
    """Run once with NTFF tracing; returns max per-core exec_time_ns or None."""
    images = np.ascontiguousarray(np.asarray(images, dtype=np.float32))
    if "nc" not in _CACHE:
        _CACHE["nc"] = _build_program()
    try:
        res = run_bass_kernel_spmd(
            _CACHE["nc"], _in_maps(images), core_ids=list(range(NCORES)),
            trace=True, trace_cores=[0],
        )
        return res.exec_time_ns
    except Exception as exc:
        print(f"timed_run: trace failed ({exc})")
        return None


if __name__ == "__main__":
    rng = np.random.default_rng(0)
    imgs = rng.random((NFRAMES, H, W), dtype=np.float32)
    o = kernel(imgs)
    print("ran:", o.shape, o.dtype, float(np.abs(o).max()))


# revision 16
# speedup vs baseline: 1.1474x; 1.1474x over previous
"""Complementary gray-code structured-light decoder on 8 Trainium2 NeuronCores.

kernel(images: [24, 2048, 2448] f32) -> [2048, 2448, 2] f32

Sharding: H rows split across 8 cores (256 rows each), data-parallel, no
cross-core communication. Per core the 256x2448 slab is processed as 12
tiles of [128 rows x 408 cols].

V2 rework of the validated baseline (sim-checked: rel_l2 2.6e-4 on CPU):
  - s, c drop the f32-einsum EPS emulation: plain subtracts (GPSIMD).
  - threshold: pairwise f32 tree (GPSIMD+DVE) instead of 8 fp32 PE matmuls;
    compares test (8*gc > sum8) via one fused scalar_tensor_tensor so the
    /8 scale op disappears.
  - reciprocal: DVE reciprocal_approx_fast (1 op) instead of full-precision
    iterative reciprocal (5.2us/tile).
  - arctan output, sela, sign planes in bf16 -> every t4 matmul is bf16.
  - XOR cascade on int32 bitcast of the {0,1.0} bf16 planes (bitwise_xor
    on 0x3F80 patterns == logical_xor, half the DVE elements).
  - final combine fused: out = (t4 + 4) * mask in one scalar_tensor_tensor
    reading PSUM (drops the +4*ones matmuls).

Math per pixel (both directions d):
  s = i1-i3, c = i0-i2 ; thr8 = treesum(i0..i7) ; bit_i = (8*gc_i > thr8)
  x_i = bit_0 ^..^ bit_i ; q = s^2+c^2 ; mask = max(q_col,q_row) > T_EFF
  u = s*c/max(s^2,c^2) ; sel = s^2>c^2 ; t4 = -S*atan(u) + 2S*sel*atan(u)
      - 4*sel*sgn(u) + sum_i 2^(10-i) x_i + 8 x_7 + 8 x_7 sgn(c) - 4 sgn(c)
  out_d = (t4 + 4) * mask * 1   [S = 16/2pi; collapsed unwrap identity]
"""
import numpy as np

import concourse.bass as bass
import concourse.mybir as mybir
import concourse.tile as tile
from concourse.vector_clock import ScopedClock
from concourse.bass_utils import run_bass_kernel_spmd

# ---------------- constants ----------------
H, W = 2048, 2448
NFRAMES = 24
NCORES = 8
ROWS_PER_CORE = H // NCORES          # 256
F = 408                              # tile free width; W = 6*F
T_EFF = 0.010000010952353477         # (q > T_EFF) == (0.5*sqrt(q) > 0.05f)
SCALE = float(np.float32(16.0 / (2.0 * np.float64(np.pi))))
WB_VALS = [-SCALE, 2.0, 1024.0, 512.0, 256.0, 128.0, 64.0, 32.0,
           16.0, 8.0, -4.0]
W_NSC, W_P2 = 0, 1
W_XI0 = 2                            # slots 2..8 hold 1024..16 for x_0..x_6
W_P8, W_N4 = 9, 10
RCP_MAGIC = 2130252226               # 0x7EF311C2: seed = MAGIC - bits(x)

f32 = mybir.dt.float32
bf16 = mybir.dt.bfloat16
i32 = mybir.dt.int32
OP = mybir.AluOpType
AF = mybir.ActivationFunctionType

_ctr = [0]


def _sanitize_waits(nc):
    """This walrus build rejects instructions carrying >1 sync wait. Move
    excess waits onto fresh same-engine NOPs inserted just before."""
    for f in nc.m.functions:
        for bb in f.blocks:
            il = bb.instructions
            i = 0
            while i < len(il):
                ins = il[i]
                si = getattr(ins, "sync_info", None)
                waits = list(si.on_wait) if si is not None and si.on_wait else []
                if len(waits) > 1:
                    si.on_wait = [waits[-1]]
                    ins.sync_info = si
                    for w in waits[:-1]:
                        _ctr[0] += 1
                        n = mybir.InstNoOp(name=f"waitsplit_{_ctr[0]}")
                        n.engine = ins.engine
                        n.sync_info = mybir.SyncInfo(on_wait=[w], on_update=[])
                        il.insert(i, n)
                        i += 1
                i += 1


class _SafeTileContext(tile.TileContext):
    """TileContext whose exit drain splits its sem waits across SP NOPs
    (the drain is emitted inside __exit__, before _sanitize_waits can run)."""

    def _drain_and_barrier(self, tick_clock, wait_clock):
        nop_inst = self.nc.sync.nop()
        wait_clock.add_sem_waits(
            nop_inst.ins, ScopedClock({None: tick_clock.global_clock})
        )
        si = nop_inst.ins.sync_info
        waits = list(si.on_wait) if si is not None else []
        if len(waits) > 1:
            si.on_wait = waits[:1]
            nop_inst.ins.sync_info = si
            for w in waits[1:]:
                n2 = self.nc.sync.nop()
                n2.ins.sync_info = mybir.SyncInfo(on_wait=[w], on_update=[])
        self.nc.sync.drain()

        self.nc.all_engine_barrier()
        assert self.sems is not None
        popped = self.nc._tile_sem_poison_stack.pop()
        assert popped is self._sem_poison
        self.nc.clear_and_free_semaphores(list(self.sems.allocated().values()))
        self.nc.all_engine_barrier()


def _build_program(sanitize=True):
    import contextlib

    nc = bass.Bass("TRN2", target_bir_lowering=False, debug=False)
    img = nc.dram_tensor("img", [NFRAMES, ROWS_PER_CORE, W], f32, kind="ExternalInput")
    wts = nc.dram_tensor("wts", [1, 128, 128], f32, kind="ExternalInput")
    wtsb = nc.dram_tensor("wtsb", [len(WB_VALS), 128, 128], bf16, kind="ExternalInput")
    out = nc.dram_tensor("out", [ROWS_PER_CORE, W, 2], f32, kind="ExternalOutput")

    with _SafeTileContext(nc) as tc, contextlib.ExitStack() as ctx:
        wpool = ctx.enter_context(tc.tile_pool(name="wpool", bufs=1))
        inp = ctx.enter_context(tc.tile_pool(name="inp", bufs=2))
        sb = ctx.enter_context(tc.tile_pool(name="sb", bufs=1))
        outp = ctx.enter_context(tc.tile_pool(name="outp", bufs=2))
        ps_t = ctx.enter_context(tc.tile_pool(name="ps_t", bufs=2, space="PSUM"))
        ps_s = ctx.enter_context(tc.tile_pool(name="ps_s", bufs=2, space="PSUM"))

        wtb = wpool.tile([128, len(WB_VALS) * 128], bf16, tag="wtb")
        for wi in range(len(WB_VALS)):
            nc.sync.dma_start(out=wtb[:, wi * 128:(wi + 1) * 128], in_=wtsb[wi, :, :])
        wtf = wpool.tile([128, 128], f32, tag="wtf")
        nc.sync.dma_start(out=wtf[:, :], in_=wts[0, :, :])
        cmagic = wpool.tile([128, 1], i32, tag="cmagic")
        nc.vector.memset(cmagic[:, :], RCP_MAGIC)

        def wb(i):
            return wtb[:, i * 128:(i + 1) * 128]

        for rb in range(ROWS_PER_CORE // 128):
            r0 = rb * 128
            for cb in range(W // F):
                c0 = cb * F
                # ---------------- load ----------------
                X = inp.tile([128, NFRAMES * F], f32, tag="X")
                nc.sync.dma_start(
                    out=X[:, :].rearrange("p (f x) -> p f x", f=NFRAMES),
                    in_=img[:, r0:r0 + 128, c0:c0 + F].rearrange("f p x -> p f x"),
                )
                # PS frames as [2, 4, F]: [:, j, :] = frame-pair (j, j+4)
                Xp = X[:, 0:8 * F].rearrange("p (a b x) -> p a b x", a=2, b=4)
                # gray frames as [d, bit, x]
                Xg = X[:, 8 * F:24 * F].rearrange("p (d f x) -> p d f x", d=2, f=8)

                # ---------------- s, c (GPSIMD) ----------------
                # cs layout: [c_col | s_col | c_row | s_row]
                cs = sb.tile([128, 4 * F], f32, tag="cs", bufs=2)
                csv = cs[:, :].rearrange("p (g t y) -> p g t y", g=2, t=2)
                c_view = csv[:, :, 0, :]
                s_view = csv[:, :, 1, :]
                nc.gpsimd.tensor_tensor(c_view, Xp[:, :, 0, :], Xp[:, :, 2, :], OP.subtract)
                nc.gpsimd.tensor_tensor(s_view, Xp[:, :, 1, :], Xp[:, :, 3, :], OP.subtract)

                # ---------------- threshold sum (PE, f32 PSUM accumulate) ---
                thr8 = ps_s.tile([128, F], f32, tag="thr8")
                for i in range(8):
                    nc.tensor.matmul(thr8[:, :], wtf[:, :], X[:, i * F:(i + 1) * F],
                                     start=(i == 0), stop=(i == 7))
                thr8b = thr8[:, :].rearrange("p (o v x) -> p o v x", o=1, v=1)

                # ---------------- gray compares (fused scale) ----------------
                # xbits layout: [x0 pair | x7 pair | sel pair]  (pair = col,row)
                xbits = sb.tile([128, 6 * F], bf16, tag="xbits", bufs=2)
                nc.vector.scalar_tensor_tensor(
                    xbits[:, 0:2 * F].rearrange("p (d x) -> p d x", d=2),
                    Xg[:, :, 0, :], 8.0,
                    thr8[:, :].rearrange("p (o x) -> p o x", o=1).broadcast_to([128, 2, F]),
                    OP.mult, OP.is_gt,
                )
                b_rest = sb.tile([128, 14 * F], bf16, tag="b_rest")
                brv = b_rest[:, :].rearrange("p (f d x) -> p d f x", f=7, d=2)
                for d in range(2):
                    nc.vector.scalar_tensor_tensor(
                        brv[:, d, :, :], Xg[:, d, 1:8, :], 8.0,
                        thr8b[:, 0, :, :].broadcast_to([128, 7, F]),
                        OP.mult, OP.is_gt,
                    )

                # ---------------- XOR cascade (int32 alias) ----------------
                x_all = sb.tile([128, 12 * F], bf16, tag="x_all", bufs=2)

                def xpair(i):  # contiguous [128, 2F] cumulative bit i (pair)
                    if i == 0:
                        return xbits[:, 0:2 * F]
                    if i == 7:
                        return xbits[:, 2 * F:4 * F]
                    return x_all[:, (i - 1) * 2 * F:i * 2 * F]

                for i in range(1, 8):
                    nc.vector.tensor_tensor(
                        xpair(i).bitcast(i32), xpair(i - 1).bitcast(i32),
                        b_rest[:, (i - 1) * 2 * F:i * 2 * F].bitcast(i32),
                        OP.bitwise_xor,
                    )

                def x_dir(i, d):
                    return xpair(i)[:, d * F:(d + 1) * F]

                # ---------------- squares, magnitude mask ----------------
                sq_s = sb.tile([128, 2 * F], f32, tag="sq_s")
                sq_c = sb.tile([128, 2 * F], f32, tag="sq_c")
                nc.scalar.activation(
                    sq_s[:, :].rearrange("p (d x) -> p d x", d=2),
                    s_view, AF.Square, bias=0.0, scale=1.0,
                )
                nc.scalar.activation(
                    sq_c[:, :].rearrange("p (d x) -> p d x", d=2),
                    c_view, AF.Square, bias=0.0, scale=1.0,
                )
                q_all = sb.tile([128, 2 * F], f32, tag="q_all")
                nc.gpsimd.tensor_tensor(q_all[:, :], sq_s[:, :], sq_c[:, :], OP.add)
                qm = sb.tile([128, F], f32, tag="qm")
                nc.vector.tensor_tensor(qm[:, :], q_all[:, 0:F], q_all[:, F:2 * F], OP.max)
                mask = sb.tile([128, F], f32, tag="mask", bufs=2)
                nc.vector.tensor_single_scalar(mask[:, :], qm[:, :], T_EFF, OP.is_gt)

                # ---------------- bounded atan path ----------------
                ma2 = sb.tile([128, 2 * F], f32, tag="ma2")
                nc.vector.tensor_tensor(ma2[:, :], sq_s[:, :], sq_c[:, :], OP.max)
                p_sc = sb.tile([128, 2 * F], f32, tag="p_sc")
                nc.gpsimd.tensor_tensor(
                    p_sc[:, :].rearrange("p (d x) -> p d x", d=2),
                    s_view, c_view, OP.mult,
                )
                # Newton reciprocal of ma2: r0 = bitcast((C-1) - bits(ma2));
                # rneg = (ma2*r0 - 2)*r0 == -1/ma2 (rel err <= 2.6e-3)
                rc0 = sb.tile([128, 2 * F], f32, tag="r0")
                nc.vector.tensor_tensor(
                    rc0[:, :].bitcast(i32),
                    cmagic[:, :].broadcast_to([128, 2 * F]),
                    ma2[:, :].bitcast(i32), OP.subtract,
                )
                tn = sb.tile([128, 2 * F], f32, tag="tn")
                nc.gpsimd.tensor_tensor(tn[:, :], ma2[:, :], rc0[:, :], OP.mult)
                rneg = sb.tile([128, 2 * F], f32, tag="rneg")
                nc.vector.scalar_tensor_tensor(
                    rneg[:, :], tn[:, :], 2.0, rc0[:, :], OP.subtract, OP.mult,
                )
                u_all = sb.tile([128, 2 * F], f32, tag="u_all")
                nc.gpsimd.tensor_tensor(u_all[:, :], p_sc[:, :], rneg[:, :], OP.mult)
                # u_all = -u  ->  a_u holds -atan(u); weights account for it
                a_u = sb.tile([128, 2 * F], bf16, tag="a_u", bufs=2)
                nc.scalar.activation(a_u[:, :], u_all[:, :], AF.Arctan, bias=0.0, scale=1.0)

                # sgn layout: [sgn_c pair | -sgn_u pair]
                sgn = sb.tile([128, 4 * F], bf16, tag="sgn", bufs=2)
                nc.scalar.activation(
                    sgn[:, 0:2 * F].rearrange("p (d x) -> p d x", d=2),
                    c_view, AF.Sign, bias=0.0, scale=1.0,
                )
                nc.scalar.activation(sgn[:, 2 * F:4 * F], p_sc[:, :], AF.Sign, bias=0.0, scale=-1.0)

                # sel2 = Sign(sq_s - sq_c) in {-1,0,1} into xbits[4F:6F]
                dsq = sb.tile([128, 2 * F], f32, tag="dsq")
                nc.gpsimd.tensor_tensor(dsq[:, :], sq_s[:, :], sq_c[:, :], OP.subtract)
                nc.scalar.activation(xbits[:, 4 * F:6 * F], dsq[:, :], AF.Sign, bias=0.0, scale=1.0)
                # A2 = sel2 * a_u ; sgnm = [x7*sgn_c | sel2*(-sgn_u)]
                sela = sb.tile([128, 2 * F], bf16, tag="sela", bufs=2)
                nc.vector.tensor_tensor(sela[:, :], xbits[:, 4 * F:6 * F], a_u[:, :], OP.mult)
                sgnm = sb.tile([128, 4 * F], bf16, tag="sgnm", bufs=2)
                nc.vector.tensor_tensor(
                    sgnm[:, :],
                    xbits[:, 2 * F:6 * F].rearrange("p (h y) -> p h y", h=2),
                    sgn[:, :].rearrange("p (h y) -> p h y", h=2),
                    OP.mult,
                )

                # ---------------- PE: t4 linear combine (all bf16) ----------
                # t4 = -S*a_u + 2S*sela - 4*sel*sgn_u + sum_i 2^(10-i)*x_i
                #      + 8*x7 + 8*x7*sgn_c - 4*sgn_c     (+4 fused into output)
                t4 = ps_t.tile([128, 1024], f32, tag="t4", name="t4")
                for d in range(2):
                    sl = slice(d * F, (d + 1) * F)
                    s2 = slice(2 * F + d * F, 2 * F + (d + 1) * F)
                    pd = t4[:, d * 512:d * 512 + F]
                    nc.tensor.matmul(pd, wb(W_NSC), sela[:, sl], start=True, stop=False)
                    nc.tensor.matmul(pd, wb(W_P2), sgn[:, s2], start=False, stop=False)
                    nc.tensor.matmul(pd, wb(W_P2), sgnm[:, s2], start=False, stop=False)
                    for i in range(7):
                        nc.tensor.matmul(pd, wb(W_XI0 + i), x_dir(i, d), start=False, stop=False)
                    nc.tensor.matmul(pd, wb(W_P8), x_dir(7, d), start=False, stop=False)
                    nc.tensor.matmul(pd, wb(W_P8), sgnm[:, sl], start=False, stop=False)
                    nc.tensor.matmul(pd, wb(W_N4), sgn[:, sl], start=False, stop=True)

                # ---------------- fused (t4+4)*mask + interleaved store -----
                o_t = outp.tile([128, F * 2], f32, tag="o_t")
                ov = o_t[:, :].rearrange("p (x two) -> p two x", two=2)
                t4v = t4[:, :].rearrange("p (g y) -> p g y", g=2)[:, :, 0:F]
                maskb = mask[:, :].rearrange("p (o x) -> p o x", o=1).broadcast_to([128, 2, F])
                nc.vector.scalar_tensor_tensor(
                    ov, t4v, 4.0, maskb, OP.add, OP.mult,
                )
                nc.sync.dma_start(
                    out=out[r0:r0 + 128, c0:c0 + F, :].rearrange("p x two -> p (x two)"),
                    in_=o_t[:, :],
                )

    if sanitize:
        _sanitize_waits(nc)
    return nc


def _weights_b():
    import ml_dtypes
    I = np.eye(128, dtype=np.float32)
    return np.stack([np.float32(v) * I for v in WB_VALS]).astype(ml_dtypes.bfloat16)


_CACHE = {}


def _in_maps(images):
    wtsb = _weights_b()
    wts = np.eye(128, dtype=np.float32)[None]
    maps = []
    for core in range(NCORES):
        r0 = core * ROWS_PER_CORE
        maps.append({
            "img": np.ascontiguousarray(images[:, r0:r0 + ROWS_PER_CORE, :]),
            "wts": wts,
            "wtsb": wtsb,
        })
    return maps


def kernel(images: np.ndarray) -> np.ndarray:
    images = np.ascontiguousarray(np.asarray(images, dtype=np.float32))
    assert images.shape == (NFRAMES, H, W), images.shape
    if "nc" not in _CACHE:
        _CACHE["nc"] = _build_program()
    res = run_bass_kernel_spmd(_CACHE["nc"], _in_maps(images), core_ids=list(range(NCORES)))
    out = np.empty((H, W, 2), dtype=np.float32)
    for core in range(NCORES):
        r0 = core * ROWS_PER_CORE
        out[r0:r0 + ROWS_PER_CORE] = res.results[core]["out"]
    return out


def timed_run(images: np.ndarray):
    """Run once with NTFF tracing; returns max per-core exec_time_ns or None."""
    images = np.ascontiguousarray(np.asarray(images, dtype=np.float32))
    if "nc" not in _CACHE:
        _CACHE["nc"] = _build_program()
    try:
        res = run_bass_kernel_spmd(
            _CACHE["nc"], _in_maps(images), core_ids=list(range(NCORES)),
            trace=True, trace_cores=[0],
        )
        return res.exec_time_ns
    except Exception as exc:
        print(f"timed_run: trace failed ({exc})")
        return None


if __name__ == "__main__":
    rng = np.random.default_rng(0)
    imgs = rng.random((NFRAMES, H, W), dtype=np.float32)
    o = kernel(imgs)
    print("ran:", o.shape, o.dtype, float(np.abs(o).max()))
